# revision 3
# baseline (speedup 1.0000x reference)
"""BidLatte (linear-attention) Trainium2 kernel, 8-core SPMD.

Math (per batch b):
  K = X@Wk; Q = X@Wq; E = exp(K)*mask          (max-shift cancels exactly)
  Ksum = sum_t E;  KX = E^T @ X                (L x D state, avoids X@Wv)
  KXn = KX / Ksum; Kv = KXn @ Wv; Kv_bd = blockdiag_head(Kv)
  G = Kv_bd @ o_proj                           (o_proj folded into state)
  out = softmax_head(Q) @ G

Sharding: core 2i+j -> batch i, T-half j. One pairwise AllReduce of the
(L x D+1) state per batch pair. Matmuls run in float32r (TF32-like, full
PE rate at N>=256).
"""
import numpy as np

_B, _T, _D, _L, _H = 4, 8192, 1024, 128, 16
NCORES = 8
TLOC = _T // 2  # tokens per core
BT = 512        # tokens per block
NBLK = TLOC // BT
NT = BT // 128  # t-tiles per block
DC = _D // 128  # d-chunks

_cache = {}


def _build():
    import concourse.bacc as bacc
    import concourse.mybir as mybir
    import concourse.tile as tile

    FP32 = mybir.dt.float32
    FP32R = mybir.dt.float32r
    EXP = mybir.ActivationFunctionType.Exp

    nc = bacc.Bacc("TRN2", target_bir_lowering=False, debug=False,
                   num_devices=NCORES)

    xs = nc.dram_tensor("xs", [TLOC, _D], FP32R, kind="ExternalInput")
    ms = nc.dram_tensor("ms", [128, TLOC // 128], FP32, kind="ExternalInput")
    wk = nc.dram_tensor("wk", [128, _D], FP32R, kind="ExternalInput")
    wq = nc.dram_tensor("wq", [128, _D], FP32R, kind="ExternalInput")
    wv = nc.dram_tensor("wv", [128, DC * _D], FP32R, kind="ExternalInput")
    op = nc.dram_tensor("op", [128, DC * _D], FP32R, kind="ExternalInput")
    ident = nc.dram_tensor("ident", [128, 128], FP32R, kind="ExternalInput")
    ph = nc.dram_tensor("ph", [128, _H], FP32R, kind="ExternalInput")
    pht = nc.dram_tensor("pht", [_H, 128], FP32R, kind="ExternalInput")
    ones2 = nc.dram_tensor("ones2", [128, 2], FP32R, kind="ExternalInput")
    bdm = nc.dram_tensor("bdm", [128, _D], FP32, kind="ExternalInput")
    out = nc.dram_tensor("out", [TLOC, _D], FP32, kind="ExternalOutput")

    with tile.TileContext(nc) as tc:
        with (
            tc.tile_pool(name="const", bufs=1) as cpool,
            tc.tile_pool(name="dram", bufs=1, space="DRAM") as dpool,
        ):
            wk_sb = cpool.tile([128, _D], FP32R)
            wq_sb = cpool.tile([128, _D], FP32R)
            wv_sb = cpool.tile([128, DC * _D], FP32R)
            op_sb = cpool.tile([128, DC * _D], FP32R)
            id_sb = cpool.tile([128, 128], FP32R)
            ph_sb = cpool.tile([128, _H], FP32R)
            pht_sb = cpool.tile([_H, 128], FP32R)
            on_sb = cpool.tile([128, 2], FP32R)
            ms_sb = cpool.tile([128, TLOC // 128], FP32)
            qst_sb = cpool.tile([128, TLOC], FP32R)   # persistent softmax(Q)^T
            kxp_sb = cpool.tile([128, 1032], FP32)    # packed KX | Ksum
            kxr_sb = cpool.tile([128, 1032], FP32)    # reduced state
            g_sb = cpool.tile([128, _D], FP32R)       # folded output weights

            nc.sync.dma_start(out=wk_sb[:], in_=wk.ap())
            nc.sync.dma_start(out=wq_sb[:], in_=wq.ap())
            nc.sync.dma_start(out=wv_sb[:], in_=wv.ap())
            nc.sync.dma_start(out=op_sb[:], in_=op.ap())
            nc.sync.dma_start(out=id_sb[:], in_=ident.ap())
            nc.sync.dma_start(out=ph_sb[:], in_=ph.ap())
            nc.sync.dma_start(out=pht_sb[:], in_=pht.ap())
            nc.sync.dma_start(out=on_sb[:], in_=ones2.ap())
            nc.sync.dma_start(out=ms_sb[:], in_=ms.ap())
            bdm_sb = cpool.tile([128, _D], FP32)
            nc.sync.dma_start(out=bdm_sb[:], in_=bdm.ap())

            ar_in = dpool.tile([128, 1032], FP32)
            ar_out = dpool.tile([128, 1032], FP32)

            # ---------------- Phase A: state + softmax(Q)^T ----------------
            with (
                tc.tile_pool(name="xin", bufs=8) as xin,
                tc.tile_pool(name="xt", bufs=3) as xtp,
                tc.tile_pool(name="esb", bufs=2) as esb,
                tc.tile_pool(name="e2", bufs=8) as e2p,
                tc.tile_pool(name="srp", bufs=2) as srp,
                tc.tile_pool(name="scr_ps", bufs=3, space="PSUM") as scr,
                tc.tile_pool(name="kt_ps", bufs=1, space="PSUM") as ktp,
                tc.tile_pool(name="qt_ps", bufs=1, space="PSUM") as qtp,
                tc.tile_pool(name="kx_ps", bufs=1, space="PSUM") as kxp,
                tc.tile_pool(name="ks_ps", bufs=1, space="PSUM") as ksp,
            ):
                kx_ps = kxp.tile([128, _D], FP32)
                ks_ps = ksp.tile([128, 2], FP32)
                for k in range(NBLK):
                    xts = []
                    for i in range(NT):
                        xt_in = xin.tile([128, _D], FP32R, tag="xin")
                        r0 = k * BT + i * 128
                        nc.sync.dma_start(out=xt_in[:], in_=xs.ap()[r0:r0 + 128, :])
                        xts.append(xt_in)

                    kt_ps = ktp.tile([128, BT], FP32)
                    qt_ps = qtp.tile([128, BT], FP32)
                    for c in range(DC):
                        tp = scr.tile([128, BT], FP32R, tag="scr")
                        for i in range(NT):
                            nc.tensor.transpose(
                                tp[:, i * 128:(i + 1) * 128],
                                xts[i][:, c * 128:(c + 1) * 128],
                                id_sb[:],
                            )
                        xt = xtp.tile([128, BT], FP32R, tag="xt")
                        if c % 2 == 0:
                            nc.vector.tensor_copy(xt[:], tp[:])
                        else:
                            nc.scalar.copy(xt[:], tp[:])
                        nc.tensor.matmul(
                            kt_ps[:], wk_sb[:, c * 128:(c + 1) * 128], xt[:],
                            start=(c == 0), stop=(c == DC - 1),
                        )
                        nc.tensor.matmul(
                            qt_ps[:], wq_sb[:, c * 128:(c + 1) * 128], xt[:],
                            start=(c == 0), stop=(c == DC - 1),
                        )

                    # E^T = exp(K^T), expQ^T = exp(Q^T)
                    et = esb.tile([128, BT], FP32R, tag="et")
                    nc.scalar.activation(et[:], kt_ps[:], EXP)
                    eq = esb.tile([128, BT], FP32R, tag="eq")
                    nc.scalar.activation(eq[:], qt_ps[:], EXP)

                    # softmax over head groups (partition dim) via pool-matmuls
                    s_ps = scr.tile([_H, BT], FP32, tag="scr")
                    nc.tensor.matmul(s_ps[:], ph_sb[:], eq[:], start=True,
                                     stop=True)
                    sr = srp.tile([_H, BT], FP32R, tag="sr")
                    with nc.allow_low_precision(reason="f32r recip, 1e-4 ok"):
                        nc.vector.reciprocal(sr[:], s_ps[:])
                    bq_ps = scr.tile([128, BT], FP32, tag="scr")
                    nc.tensor.matmul(bq_ps[:], pht_sb[:], sr[:], start=True,
                                     stop=True)
                    nc.vector.tensor_mul(
                        qst_sb[:, k * BT:(k + 1) * BT], eq[:], bq_ps[:]
                    )

                    # transpose E^T back to [t, L], mask, accumulate KX/Ksum
                    e_ps = scr.tile([128, BT], FP32R, tag="scr")
                    for i in range(NT):
                        nc.tensor.transpose(
                            e_ps[:, i * 128:(i + 1) * 128],
                            et[:, i * 128:(i + 1) * 128],
                            id_sb[:],
                        )
                    for i in range(NT):
                        e2 = e2p.tile([128, 128], FP32R, tag="e2")
                        j = k * NT + i
                        nc.vector.tensor_scalar_mul(
                            e2[:], e_ps[:, i * 128:(i + 1) * 128],
                            ms_sb[:, j:j + 1],
                        )
                        first = (k == 0 and i == 0)
                        last = (k == NBLK - 1 and i == NT - 1)
                        nc.tensor.matmul(kx_ps[:, 0:512], e2[:],
                                         xts[i][:, 0:512],
                                         start=first, stop=last)
                        nc.tensor.matmul(kx_ps[:, 512:1024], e2[:],
                                         xts[i][:, 512:1024],
                                         start=first, stop=last)
                        nc.tensor.matmul(ks_ps[:], e2[:], on_sb[:],
                                         start=first, stop=last)

                # pack state for the collective
                nc.vector.tensor_copy(kxp_sb[:, 0:512], kx_ps[:, 0:512])
                nc.scalar.copy(kxp_sb[:, 512:1024], kx_ps[:, 512:1024])
                nc.vector.tensor_copy(kxp_sb[:, 1024:1025], ks_ps[:, 0:1])
                nc.vector.memset(kxp_sb[:, 1025:1032], 0.0)

            nc.sync.dma_start(out=ar_in[:], in_=kxp_sb[:])
            nc.gpsimd.collective_compute(
                "AllReduce",
                mybir.AluOpType.add,
                replica_groups=[[0, 1], [2, 3], [4, 5], [6, 7]],
                ins=[ar_in.opt()],
                outs=[ar_out.opt()],
            )
            nc.sync.dma_start(out=kxr_sb[:], in_=ar_out[:])

            # ---------------- Phase B: G = blockdiag(KXn @ Wv) @ o_proj ----
            with (
                tc.tile_pool(name="bsb", bufs=2) as bsb,
                tc.tile_pool(name="bsb1", bufs=1) as bsb1,
                tc.tile_pool(name="bps_small", bufs=2, space="PSUM") as bpss,
                tc.tile_pool(name="bps_big", bufs=2, space="PSUM") as bpsb,
            ):
                rk = bsb1.tile([128, 1], FP32)
                nc.vector.reciprocal(rk[:], kxr_sb[:, 1024:1025])
                kxn = bsb1.tile([128, _D], FP32R)
                nc.vector.tensor_scalar_mul(kxn[:], kxr_sb[:, 0:1024], rk[:])

                kxnt = []
                for c in range(DC):
                    tp = bpss.tile([128, 128], FP32R, tag="btp")
                    nc.tensor.transpose(tp[:], kxn[:, c * 128:(c + 1) * 128],
                                        id_sb[:])
                    t_sb = bsb.tile([128, 128], FP32R, tag="bts")
                    nc.vector.tensor_copy(t_sb[:], tp[:])
                    kxnt.append(t_sb)

                kv_ps = bpsb.tile([128, _D], FP32, tag="big")
                for c in range(DC):
                    nc.tensor.matmul(
                        kv_ps[:, 0:512], kxnt[c][:],
                        wv_sb[:, c * _D:c * _D + 512],
                        start=(c == 0), stop=(c == DC - 1))
                    nc.tensor.matmul(
                        kv_ps[:, 512:1024], kxnt[c][:],
                        wv_sb[:, c * _D + 512:(c + 1) * _D],
                        start=(c == 0), stop=(c == DC - 1))

                # block-diagonal extract via 0/1 mask multiply (f32)
                kvbd = bsb1.tile([128, _D], FP32)
                nc.vector.tensor_mul(kvbd[:], kv_ps[:], bdm_sb[:])
                kvbdt = []
                for c in range(DC):
                    tp = bpss.tile([128, 128], FP32, tag="btpf")
                    nc.tensor.transpose(tp[:], kvbd[:, c * 128:(c + 1) * 128],
                                        id_sb[:].bitcast(FP32))
                    t_sb = bsb.tile([128, 128], FP32R, tag="btsf")
                    nc.vector.tensor_copy(t_sb[:], tp[:])
                    kvbdt.append(t_sb)

                g_ps = bpsb.tile([128, _D], FP32, tag="big")
                for c in range(DC):
                    nc.tensor.matmul(
                        g_ps[:, 0:512], kvbdt[c][:],
                        op_sb[:, c * _D:c * _D + 512],
                        start=(c == 0), stop=(c == DC - 1))
                    nc.tensor.matmul(
                        g_ps[:, 512:1024], kvbdt[c][:],
                        op_sb[:, c * _D + 512:(c + 1) * _D],
                        start=(c == 0), stop=(c == DC - 1))
                nc.vector.tensor_copy(g_sb[:, 0:512], g_ps[:, 0:512])
                nc.scalar.copy(g_sb[:, 512:1024], g_ps[:, 512:1024])

            # ---------------- Phase C: out = Qs @ G ------------------------
            with (
                tc.tile_pool(name="osb", bufs=3) as osb,
                tc.tile_pool(name="ops", bufs=2, space="PSUM") as ops,
            ):
                for i in range(TLOC // 128):
                    o_ps = ops.tile([128, _D], FP32, tag="ops")
                    lhs = qst_sb[:, i * 128:(i + 1) * 128]
                    nc.tensor.matmul(o_ps[:, 0:512], lhs, g_sb[:, 0:512],
                                     start=True, stop=True)
                    nc.tensor.matmul(o_ps[:, 512:1024], lhs,
                                     g_sb[:, 512:1024], start=True, stop=True)
                    ot = osb.tile([128, _D], FP32, tag="osb")
                    if i % 2 == 0:
                        nc.vector.tensor_copy(ot[:], o_ps[:])
                    else:
                        nc.scalar.copy(ot[:], o_ps[:])
                    nc.sync.dma_start(out=out.ap()[i * 128:(i + 1) * 128, :],
                                      in_=ot[:])

    nc.compile()
    return nc


def _host_inputs(X, attention_mask, Wk, Wq, Wv, o_proj):
    X = np.ascontiguousarray(np.asarray(X, dtype=np.float32))
    mask = np.asarray(attention_mask, dtype=np.float32)
    Wk = np.asarray(Wk, dtype=np.float32)
    Wq = np.asarray(Wq, dtype=np.float32)
    Wv = np.asarray(Wv, dtype=np.float32)
    o_proj = np.asarray(o_proj, dtype=np.float32)

    wk_r = np.ascontiguousarray(
        Wk.reshape(DC, 128, _L).transpose(1, 0, 2).reshape(128, DC * _L))
    wq_r = np.ascontiguousarray(
        Wq.reshape(DC, 128, _L).transpose(1, 0, 2).reshape(128, DC * _L))
    wv_r = np.ascontiguousarray(
        Wv.reshape(DC, 128, _D).transpose(1, 0, 2).reshape(128, DC * _D))
    op_r = np.ascontiguousarray(
        o_proj.reshape(DC, 128, _D).transpose(1, 0, 2).reshape(128, DC * _D))
    ident = np.eye(128, dtype=np.float32)
    ph_m = np.zeros((128, _H), dtype=np.float32)
    for hh in range(_H):
        ph_m[hh * (_L // _H):(hh + 1) * (_L // _H), hh] = 1.0
    pht_m = np.ascontiguousarray(ph_m.T)
    ones2 = np.ones((128, 2), dtype=np.float32)
    bdm_m = np.zeros((128, _D), dtype=np.float32)
    for hh in range(_H):
        bdm_m[hh * (_L // _H):(hh + 1) * (_L // _H),
              hh * (_D // _H):(hh + 1) * (_D // _H)] = 1.0

    in_maps = []
    for core in range(NCORES):
        b, half = core // 2, core % 2
        xsh = np.ascontiguousarray(X[b, half * TLOC:(half + 1) * TLOC, :])
        msh = np.ascontiguousarray(
            mask[b, half * TLOC:(half + 1) * TLOC]
            .reshape(TLOC // 128, 128).T)
        in_maps.append({
            "xs": xsh, "ms": msh, "wk": wk_r, "wq": wq_r, "wv": wv_r,
            "op": op_r, "ident": ident, "ph": ph_m, "pht": pht_m,
            "ones2": ones2, "bdm": bdm_m,
        })
    return in_maps


def _run(in_maps, trace=False):
    from concourse.bass_utils import run_bass_kernel_spmd

    if "nc" not in _cache:
        _cache["nc"] = _build()
    return run_bass_kernel_spmd(
        _cache["nc"], in_maps, list(range(NCORES)), trace=trace)


def kernel(X, attention_mask, Wk, Wq, Wv, o_proj, n_heads=16):
    in_maps = _host_inputs(X, attention_mask, Wk, Wq, Wv, o_proj)
    res = _run(in_maps)
    out = np.empty((_B, _T, _D), dtype=np.float32)
    for core in range(NCORES):
        b, half = core // 2, core % 2
        out[b, half * TLOC:(half + 1) * TLOC, :] = res.results[core]["out"]
    return out


# revision 4
# speedup vs baseline: 1.1848x; 1.1848x over previous
"""BidLatte (linear-attention) Trainium2 kernel, 8-core SPMD.

Math (per batch b):
  K = X@Wk; Q = X@Wq; E = exp(K)*mask          (max-shift cancels exactly)
  Ksum = sum_t E;  KX = E^T @ X                (L x D state, avoids X@Wv)
  KXn = KX / Ksum; Kv = KXn @ Wv; Kv_bd = blockdiag_head(Kv)
  G = Kv_bd @ o_proj                           (o_proj folded into state)
  out = softmax_head(Q) @ G

Sharding: core 2i+j -> batch i, T-half j. One pairwise AllReduce of the
(L x D+1) state per batch pair. Matmuls run in float32r (TF32-like, full
PE rate at N>=256).
"""
import numpy as np

_B, _T, _D, _L, _H = 4, 8192, 1024, 128, 16
NCORES = 8
TLOC = _T // 2  # tokens per core
BT = 512        # tokens per block
NBLK = TLOC // BT
NT = BT // 128  # t-tiles per block
DC = _D // 128  # d-chunks

_cache = {}


def _build():
    import concourse.bacc as bacc
    import concourse.mybir as mybir
    import concourse.tile as tile

    FP32 = mybir.dt.float32
    FP32R = mybir.dt.float32r
    EXP = mybir.ActivationFunctionType.Exp

    nc = bacc.Bacc("TRN2", target_bir_lowering=False, debug=False,
                   num_devices=NCORES)

    xs = nc.dram_tensor("xs", [TLOC, _D], FP32R, kind="ExternalInput")
    ms = nc.dram_tensor("ms", [128, TLOC // 128], FP32, kind="ExternalInput")
    wk = nc.dram_tensor("wk", [128, _D], FP32R, kind="ExternalInput")
    wq = nc.dram_tensor("wq", [128, _D], FP32R, kind="ExternalInput")
    wv = nc.dram_tensor("wv", [128, DC * _D], FP32R, kind="ExternalInput")
    op = nc.dram_tensor("op", [128, DC * _D], FP32R, kind="ExternalInput")
    ident = nc.dram_tensor("ident", [128, 128], FP32R, kind="ExternalInput")
    ph = nc.dram_tensor("ph", [128, _H], FP32R, kind="ExternalInput")
    pht = nc.dram_tensor("pht", [_H, 128], FP32, kind="ExternalInput")
    ones2 = nc.dram_tensor("ones2", [128, 2], FP32R, kind="ExternalInput")
    bdm = nc.dram_tensor("bdm", [128, _D], FP32, kind="ExternalInput")
    out = nc.dram_tensor("out", [TLOC, _D], FP32, kind="ExternalOutput")

    with tile.TileContext(nc) as tc:
        with (
            tc.tile_pool(name="const", bufs=1) as cpool,
            tc.tile_pool(name="dram", bufs=1, space="DRAM") as dpool,
        ):
            wk_sb = cpool.tile([128, _D], FP32R)
            wq_sb = cpool.tile([128, _D], FP32R)
            wv_sb = cpool.tile([128, DC * _D], FP32R)
            op_sb = cpool.tile([128, DC * _D], FP32R)
            id_sb = cpool.tile([128, 128], FP32R)
            ph_sb = cpool.tile([128, _H], FP32R)
            pht_sb = cpool.tile([_H, 128], FP32)
            on_sb = cpool.tile([128, 2], FP32R)
            ms_sb = cpool.tile([128, TLOC // 128], FP32)
            qst_sb = cpool.tile([128, TLOC], FP32R)   # persistent softmax(Q)^T
            kxp_sb = cpool.tile([128, 1032], FP32)    # packed KX | Ksum
            kxr_sb = cpool.tile([128, 1032], FP32)    # reduced state
            g_sb = cpool.tile([128, _D], FP32R)       # folded output weights

            nc.sync.dma_start(out=wk_sb[:], in_=wk.ap())
            nc.sync.dma_start(out=wq_sb[:], in_=wq.ap())
            nc.sync.dma_start(out=id_sb[:], in_=ident.ap())
            nc.sync.dma_start(out=ph_sb[:], in_=ph.ap())
            nc.sync.dma_start(out=pht_sb[:], in_=pht.ap())
            nc.sync.dma_start(out=on_sb[:], in_=ones2.ap())
            nc.sync.dma_start(out=ms_sb[:], in_=ms.ap())
            bdm_sb = cpool.tile([128, _D], FP32)
            nc.sync.dma_start(out=bdm_sb[:], in_=bdm.ap())

            ar_in = dpool.tile([128, 1032], FP32)
            ar_out = dpool.tile([128, 1032], FP32)

            # warm up the collectives stack with a tiny dummy AllReduce
            warm_sb = cpool.tile([128, 8], FP32)
            nc.vector.memset(warm_sb[:], 0.0)
            warm_in = dpool.tile([128, 8], FP32)
            warm_out = dpool.tile([128, 8], FP32)
            nc.sync.dma_start(out=warm_in[:], in_=warm_sb[:])
            nc.gpsimd.collective_compute(
                "AllReduce",
                mybir.AluOpType.add,
                replica_groups=[[0, 1], [2, 3], [4, 5], [6, 7]],
                ins=[warm_in.opt()],
                outs=[warm_out.opt()],
            )

            # ---------------- Phase A: state + softmax(Q)^T ----------------
            with (
                tc.tile_pool(name="xin", bufs=8) as xin,
                tc.tile_pool(name="xt", bufs=3) as xtp,
                tc.tile_pool(name="esb", bufs=2) as esb,
                tc.tile_pool(name="e2", bufs=8) as e2p,
                tc.tile_pool(name="srp", bufs=2) as srp,
                tc.tile_pool(name="scr_ps", bufs=3, space="PSUM") as scr,
                tc.tile_pool(name="kt_ps", bufs=1, space="PSUM") as ktp,
                tc.tile_pool(name="qt_ps", bufs=1, space="PSUM") as qtp,
                tc.tile_pool(name="kx_ps", bufs=1, space="PSUM") as kxp,
                tc.tile_pool(name="ks_ps", bufs=1, space="PSUM") as ksp,
            ):
                kx_ps = kxp.tile([128, _D], FP32)
                ks_ps = ksp.tile([128, 2], FP32)
                for k in range(NBLK):
                    if k == 1:
                        nc.sync.dma_start(out=wv_sb[:], in_=wv.ap())
                    if k == 2:
                        nc.sync.dma_start(out=op_sb[:], in_=op.ap())
                    xts = []
                    for i in range(NT):
                        xt_in = xin.tile([128, _D], FP32R, tag="xin")
                        r0 = k * BT + i * 128
                        nc.sync.dma_start(out=xt_in[:], in_=xs.ap()[r0:r0 + 128, :])
                        xts.append(xt_in)

                    kt_ps = ktp.tile([128, BT], FP32)
                    qt_ps = qtp.tile([128, BT], FP32)
                    # software-pipelined: transposes of chunk c overlap the
                    # K/Q matmuls of chunk c-1 so the PE sees a steady
                    # matmul stream (keeps HAM warm)
                    xt_q = []
                    for c in range(DC):
                        tp = scr.tile([128, BT], FP32R, tag="scr")
                        for i in range(NT):
                            nc.tensor.transpose(
                                tp[:, i * 128:(i + 1) * 128],
                                xts[i][:, c * 128:(c + 1) * 128],
                                id_sb[:],
                            )
                        xt = xtp.tile([128, BT], FP32R, tag="xt")
                        if c % 2 == 0:
                            nc.vector.tensor_copy(xt[:], tp[:])
                        else:
                            nc.scalar.copy(xt[:], tp[:])
                        xt_q.append(xt)
                        if c >= 1:
                            cc = c - 1
                            nc.tensor.matmul(
                                kt_ps[:], wk_sb[:, cc * 128:(cc + 1) * 128],
                                xt_q[cc][:],
                                start=(cc == 0), stop=False,
                            )
                            nc.tensor.matmul(
                                qt_ps[:], wq_sb[:, cc * 128:(cc + 1) * 128],
                                xt_q[cc][:],
                                start=(cc == 0), stop=False,
                            )
                    cc = DC - 1
                    nc.tensor.matmul(
                        kt_ps[:], wk_sb[:, cc * 128:(cc + 1) * 128],
                        xt_q[cc][:], start=False, stop=True,
                    )
                    nc.tensor.matmul(
                        qt_ps[:], wq_sb[:, cc * 128:(cc + 1) * 128],
                        xt_q[cc][:], start=False, stop=True,
                    )

                    # E^T = exp(K^T), expQ^T = exp(Q^T)
                    et = esb.tile([128, BT], FP32R, tag="et")
                    nc.scalar.activation(et[:], kt_ps[:], EXP)
                    eq = esb.tile([128, BT], FP32R, tag="eq")
                    nc.scalar.activation(eq[:], qt_ps[:], EXP)

                    # softmax over head groups (partition dim) via pool-matmuls
                    s_ps = scr.tile([_H, BT], FP32, tag="scr")
                    nc.tensor.matmul(s_ps[:], ph_sb[:], eq[:], start=True,
                                     stop=True)
                    sr = srp.tile([_H, BT], FP32, tag="sr")
                    nc.vector.reciprocal_approx_fast(sr[:], s_ps[:])
                    bq_ps = scr.tile([128, BT], FP32, tag="scr")
                    nc.tensor.matmul(bq_ps[:], pht_sb[:], sr[:], start=True,
                                     stop=True)
                    nc.vector.tensor_mul(
                        qst_sb[:, k * BT:(k + 1) * BT], eq[:], bq_ps[:]
                    )

                    # transpose E^T back to [t, L], mask, accumulate KX/Ksum
                    e_ps = scr.tile([128, BT], FP32R, tag="scr")
                    for i in range(NT):
                        nc.tensor.transpose(
                            e_ps[:, i * 128:(i + 1) * 128],
                            et[:, i * 128:(i + 1) * 128],
                            id_sb[:],
                        )
                    for i in range(NT):
                        e2 = e2p.tile([128, 128], FP32R, tag="e2")
                        j = k * NT + i
                        nc.vector.tensor_scalar_mul(
                            e2[:], e_ps[:, i * 128:(i + 1) * 128],
                            ms_sb[:, j:j + 1],
                        )
                        first = (k == 0 and i == 0)
                        last = (k == NBLK - 1 and i == NT - 1)
                        nc.tensor.matmul(kx_ps[:, 0:512], e2[:],
                                         xts[i][:, 0:512],
                                         start=first, stop=last)
                        nc.tensor.matmul(kx_ps[:, 512:1024], e2[:],
                                         xts[i][:, 512:1024],
                                         start=first, stop=last)
                        nc.tensor.matmul(ks_ps[:], e2[:], on_sb[:],
                                         start=first, stop=last)

                # pack state for the collective
                nc.vector.tensor_copy(kxp_sb[:, 0:512], kx_ps[:, 0:512])
                nc.scalar.copy(kxp_sb[:, 512:1024], kx_ps[:, 512:1024])
                nc.vector.tensor_copy(kxp_sb[:, 1024:1025], ks_ps[:, 0:1])
                nc.vector.memset(kxp_sb[:, 1025:1032], 0.0)

            nc.sync.dma_start(out=ar_in[:], in_=kxp_sb[:])
            nc.gpsimd.collective_compute(
                "AllReduce",
                mybir.AluOpType.add,
                replica_groups=[[0, 1], [2, 3], [4, 5], [6, 7]],
                ins=[ar_in.opt()],
                outs=[ar_out.opt()],
            )
            nc.sync.dma_start(out=kxr_sb[:], in_=ar_out[:])

            # ---------------- Phase B: G = blockdiag(KXn @ Wv) @ o_proj ----
            with (
                tc.tile_pool(name="bsb", bufs=2) as bsb,
                tc.tile_pool(name="bsb1", bufs=1) as bsb1,
                tc.tile_pool(name="bps_small", bufs=2, space="PSUM") as bpss,
                tc.tile_pool(name="bps_big", bufs=2, space="PSUM") as bpsb,
            ):
                rk = bsb1.tile([128, 1], FP32)
                nc.vector.reciprocal_approx_fast(rk[:], kxr_sb[:, 1024:1025])
                kxn = bsb1.tile([128, _D], FP32R)
                nc.vector.tensor_scalar_mul(kxn[:], kxr_sb[:, 0:1024], rk[:])

                kxnt = []
                for c in range(DC):
                    tp = bpss.tile([128, 128], FP32R, tag="btp")
                    nc.tensor.transpose(tp[:], kxn[:, c * 128:(c + 1) * 128],
                                        id_sb[:])
                    t_sb = bsb.tile([128, 128], FP32R, tag="bts")
                    nc.vector.tensor_copy(t_sb[:], tp[:])
                    kxnt.append(t_sb)

                kv_ps = bpsb.tile([128, _D], FP32, tag="big")
                for c in range(DC):
                    nc.tensor.matmul(
                        kv_ps[:, 0:512], kxnt[c][:],
                        wv_sb[:, c * _D:c * _D + 512],
                        start=(c == 0), stop=(c == DC - 1))
                    nc.tensor.matmul(
                        kv_ps[:, 512:1024], kxnt[c][:],
                        wv_sb[:, c * _D + 512:(c + 1) * _D],
                        start=(c == 0), stop=(c == DC - 1))

                # block-diagonal extract via 0/1 mask multiply (f32)
                kvbd = bsb1.tile([128, _D], FP32)
                nc.vector.tensor_mul(kvbd[:], kv_ps[:], bdm_sb[:])
                kvbdt = []
                for c in range(DC):
                    tp = bpss.tile([128, 128], FP32, tag="btpf")
                    nc.tensor.transpose(tp[:], kvbd[:, c * 128:(c + 1) * 128],
                                        id_sb[:].bitcast(FP32))
                    t_sb = bsb.tile([128, 128], FP32R, tag="btsf")
                    nc.vector.tensor_copy(t_sb[:], tp[:])
                    kvbdt.append(t_sb)

                g_ps = bpsb.tile([128, _D], FP32, tag="big")
                for c in range(DC):
                    nc.tensor.matmul(
                        g_ps[:, 0:512], kvbdt[c][:],
                        op_sb[:, c * _D:c * _D + 512],
                        start=(c == 0), stop=(c == DC - 1))
                    nc.tensor.matmul(
                        g_ps[:, 512:1024], kvbdt[c][:],
                        op_sb[:, c * _D + 512:(c + 1) * _D],
                        start=(c == 0), stop=(c == DC - 1))
                nc.vector.tensor_copy(g_sb[:, 0:512], g_ps[:, 0:512])
                nc.scalar.copy(g_sb[:, 512:1024], g_ps[:, 512:1024])

            # ---------------- Phase C: out = Qs @ G ------------------------
            with (
                tc.tile_pool(name="osb", bufs=3) as osb,
                tc.tile_pool(name="ops", bufs=2, space="PSUM") as ops,
            ):
                for i in range(TLOC // 128):
                    o_ps = ops.tile([128, _D], FP32, tag="ops")
                    lhs = qst_sb[:, i * 128:(i + 1) * 128]
                    nc.tensor.matmul(o_ps[:, 0:512], lhs, g_sb[:, 0:512],
                                     start=True, stop=True)
                    nc.tensor.matmul(o_ps[:, 512:1024], lhs,
                                     g_sb[:, 512:1024], start=True, stop=True)
                    ot = osb.tile([128, _D], FP32, tag="osb")
                    if i % 2 == 0:
                        nc.vector.tensor_copy(ot[:], o_ps[:])
                    else:
                        nc.scalar.copy(ot[:], o_ps[:])
                    nc.sync.dma_start(out=out.ap()[i * 128:(i + 1) * 128, :],
                                      in_=ot[:])

    nc.compile()
    return nc


def _host_inputs(X, attention_mask, Wk, Wq, Wv, o_proj):
    X = np.ascontiguousarray(np.asarray(X, dtype=np.float32))
    mask = np.asarray(attention_mask, dtype=np.float32)
    Wk = np.asarray(Wk, dtype=np.float32)
    Wq = np.asarray(Wq, dtype=np.float32)
    Wv = np.asarray(Wv, dtype=np.float32)
    o_proj = np.asarray(o_proj, dtype=np.float32)

    wk_r = np.ascontiguousarray(
        Wk.reshape(DC, 128, _L).transpose(1, 0, 2).reshape(128, DC * _L))
    wq_r = np.ascontiguousarray(
        Wq.reshape(DC, 128, _L).transpose(1, 0, 2).reshape(128, DC * _L))
    wv_r = np.ascontiguousarray(
        Wv.reshape(DC, 128, _D).transpose(1, 0, 2).reshape(128, DC * _D))
    op_r = np.ascontiguousarray(
        o_proj.reshape(DC, 128, _D).transpose(1, 0, 2).reshape(128, DC * _D))
    ident = np.eye(128, dtype=np.float32)
    ph_m = np.zeros((128, _H), dtype=np.float32)
    for hh in range(_H):
        ph_m[hh * (_L // _H):(hh + 1) * (_L // _H), hh] = 1.0
    pht_m = np.ascontiguousarray(ph_m.T)
    ones2 = np.ones((128, 2), dtype=np.float32)
    bdm_m = np.zeros((128, _D), dtype=np.float32)
    for hh in range(_H):
        bdm_m[hh * (_L // _H):(hh + 1) * (_L // _H),
              hh * (_D // _H):(hh + 1) * (_D // _H)] = 1.0

    in_maps = []
    for core in range(NCORES):
        b, half = core // 2, core % 2
        xsh = np.ascontiguousarray(X[b, half * TLOC:(half + 1) * TLOC, :])
        msh = np.ascontiguousarray(
            mask[b, half * TLOC:(half + 1) * TLOC]
            .reshape(TLOC // 128, 128).T)
        in_maps.append({
            "xs": xsh, "ms": msh, "wk": wk_r, "wq": wq_r, "wv": wv_r,
            "op": op_r, "ident": ident, "ph": ph_m, "pht": pht_m,
            "ones2": ones2, "bdm": bdm_m,
        })
    return in_maps


def _run(in_maps, trace=False):
    from concourse.bass_utils import run_bass_kernel_spmd

    if "nc" not in _cache:
        _cache["nc"] = _build()
    return run_bass_kernel_spmd(
        _cache["nc"], in_maps, list(range(NCORES)), trace=trace)


def kernel(X, attention_mask, Wk, Wq, Wv, o_proj, n_heads=16):
    in_maps = _host_inputs(X, attention_mask, Wk, Wq, Wv, o_proj)
    res = _run(in_maps)
    out = np.empty((_B, _T, _D), dtype=np.float32)
    for core in range(NCORES):
        b, half = core // 2, core % 2
        out[b, half * TLOC:(half + 1) * TLOC, :] = res.results[core]["out"]
    return out


# revision 5
# speedup vs baseline: 1.4039x; 1.1849x over previous
"""BidLatte (linear-attention) Trainium2 kernel, 8-core SPMD.

Math (per batch b):
  K = X@Wk; Q = X@Wq; E = exp(K)*mask          (max-shift cancels exactly)
  Ksum = sum_t E;  KX = E^T @ X                (L x D state, avoids X@Wv)
  KXn = KX / Ksum; Kv = KXn @ Wv; Kv_bd = blockdiag_head(Kv)
  G = Kv_bd @ o_proj                           (o_proj folded into state)
  out = softmax_head(Q) @ G

Sharding: core 2i+j -> batch i, T-half j. One pairwise AllReduce of the
(L x D+1) state per batch pair.

X is fed twice in bf16 (natural + host-pre-transposed) so no on-chip
transposition of X is needed and HBM traffic drops to ~38MB/core. The
K/Q/KX matmuls run in bf16 (their errors average out in the global
T-reduction); softmax(Q), the state normalization, G and the output
matmuls run in float32r (TF32-like).
"""
import numpy as np

_B, _T, _D, _L, _H = 4, 8192, 1024, 128, 16
NCORES = 8
TLOC = _T // 2  # tokens per core
BT = 512        # tokens per block
NBLK = TLOC // BT
NT = BT // 128  # t-tiles per block
DC = _D // 128  # d-chunks

_cache = {}


def _build():
    import concourse.bacc as bacc
    import concourse.mybir as mybir
    import concourse.tile as tile

    FP32 = mybir.dt.float32
    FP32R = mybir.dt.float32r
    BF16 = mybir.dt.bfloat16
    EXP = mybir.ActivationFunctionType.Exp

    nc = bacc.Bacc("TRN2", target_bir_lowering=False, debug=False,
                   num_devices=NCORES)

    xs = nc.dram_tensor("xs", [TLOC, _D], BF16, kind="ExternalInput")
    xst = nc.dram_tensor("xst", [_D, TLOC], BF16, kind="ExternalInput")
    ms = nc.dram_tensor("ms", [128, TLOC // 128], FP32, kind="ExternalInput")
    wk = nc.dram_tensor("wk", [128, _D], BF16, kind="ExternalInput")
    wq = nc.dram_tensor("wq", [128, _D], BF16, kind="ExternalInput")
    wv = nc.dram_tensor("wv", [128, DC * _D], FP32R, kind="ExternalInput")
    op = nc.dram_tensor("op", [128, DC * _D], FP32R, kind="ExternalInput")
    ident = nc.dram_tensor("ident", [128, 128], FP32R, kind="ExternalInput")
    ph = nc.dram_tensor("ph", [128, _H], FP32R, kind="ExternalInput")
    pht = nc.dram_tensor("pht", [_H, 128], FP32, kind="ExternalInput")
    ones2 = nc.dram_tensor("ones2", [128, 2], BF16, kind="ExternalInput")
    bdm = nc.dram_tensor("bdm", [128, _D], FP32, kind="ExternalInput")
    out = nc.dram_tensor("out", [TLOC, _D], FP32, kind="ExternalOutput")

    with tile.TileContext(nc) as tc:
        with (
            tc.tile_pool(name="const", bufs=1) as cpool,
            tc.tile_pool(name="dram", bufs=1, space="DRAM") as dpool,
        ):
            wk_sb = cpool.tile([128, _D], BF16)
            wq_sb = cpool.tile([128, _D], BF16)
            wv_sb = cpool.tile([128, DC * _D], FP32R)
            op_sb = cpool.tile([128, DC * _D], FP32R)
            id_sb = cpool.tile([128, 128], FP32R)
            ph_sb = cpool.tile([128, _H], FP32R)
            pht_sb = cpool.tile([_H, 128], FP32)
            on_sb = cpool.tile([128, 2], BF16)
            ms_sb = cpool.tile([128, TLOC // 128], FP32)
            bdm_sb = cpool.tile([128, _D], FP32)
            qst_sb = cpool.tile([128, TLOC], FP32R)   # persistent softmax(Q)^T
            kxp_sb = cpool.tile([128, 1032], FP32)    # packed KX | Ksum
            kxr_sb = cpool.tile([128, 1032], FP32)    # reduced state
            g_sb = cpool.tile([128, _D], FP32R)       # folded output weights

            nc.sync.dma_start(out=wk_sb[:], in_=wk.ap())
            nc.sync.dma_start(out=wq_sb[:], in_=wq.ap())
            nc.sync.dma_start(out=id_sb[:], in_=ident.ap())
            nc.sync.dma_start(out=ph_sb[:], in_=ph.ap())
            nc.sync.dma_start(out=pht_sb[:], in_=pht.ap())
            nc.sync.dma_start(out=on_sb[:], in_=ones2.ap())
            nc.sync.dma_start(out=ms_sb[:], in_=ms.ap())
            nc.sync.dma_start(out=bdm_sb[:], in_=bdm.ap())

            ar_in = dpool.tile([128, 1032], FP32)
            ar_out = dpool.tile([128, 1032], FP32)

            # warm up the collectives stack with a tiny dummy AllReduce
            warm_sb = cpool.tile([128, 8], FP32)
            nc.vector.memset(warm_sb[:], 0.0)
            warm_in = dpool.tile([128, 8], FP32)
            warm_out = dpool.tile([128, 8], FP32)
            nc.sync.dma_start(out=warm_in[:], in_=warm_sb[:])
            nc.gpsimd.collective_compute(
                "AllReduce",
                mybir.AluOpType.add,
                replica_groups=[[0, 1], [2, 3], [4, 5], [6, 7]],
                ins=[warm_in.opt()],
                outs=[warm_out.opt()],
            )

            # ---------------- Phase A: state + softmax(Q)^T ----------------
            with (
                tc.tile_pool(name="xin", bufs=8) as xin,
                tc.tile_pool(name="xtin", bufs=12) as xtin,
                tc.tile_pool(name="esb", bufs=2) as esb,
                tc.tile_pool(name="e2", bufs=8) as e2p,
                tc.tile_pool(name="srp", bufs=2) as srp,
                tc.tile_pool(name="scr_ps", bufs=3, space="PSUM") as scr,
                tc.tile_pool(name="kt_ps", bufs=1, space="PSUM") as ktp,
                tc.tile_pool(name="qt_ps", bufs=1, space="PSUM") as qtp,
                tc.tile_pool(name="kx_ps", bufs=1, space="PSUM") as kxp,
                tc.tile_pool(name="ks_ps", bufs=1, space="PSUM") as ksp,
            ):
                kx_ps = kxp.tile([128, _D], FP32)
                ks_ps = ksp.tile([128, 2], FP32)
                for k in range(NBLK):
                    if k == 1:
                        nc.sync.dma_start(out=wv_sb[:], in_=wv.ap())
                    if k == 2:
                        nc.sync.dma_start(out=op_sb[:], in_=op.ap())
                    xts = []
                    for i in range(NT):
                        xt_in = xin.tile([128, _D], BF16, tag="xin")
                        r0 = k * BT + i * 128
                        nc.sync.dma_start(out=xt_in[:],
                                          in_=xs.ap()[r0:r0 + 128, :])
                        xts.append(xt_in)
                    xtts = []
                    for c in range(DC):
                        xtt = xtin.tile([128, BT], BF16, tag="xtin")
                        nc.sync.dma_start(
                            out=xtt[:],
                            in_=xst.ap()[c * 128:(c + 1) * 128,
                                         k * BT:(k + 1) * BT])
                        xtts.append(xtt)

                    kt_ps = ktp.tile([128, BT], FP32)
                    qt_ps = qtp.tile([128, BT], FP32)
                    for c in range(DC):
                        nc.tensor.matmul(
                            kt_ps[:], wk_sb[:, c * 128:(c + 1) * 128],
                            xtts[c][:],
                            start=(c == 0), stop=(c == DC - 1),
                        )
                        nc.tensor.matmul(
                            qt_ps[:], wq_sb[:, c * 128:(c + 1) * 128],
                            xtts[c][:],
                            start=(c == 0), stop=(c == DC - 1),
                        )

                    # E^T = exp(K^T) (f32r), expQ^T = exp(Q^T) (f32r)
                    et = esb.tile([128, BT], FP32R, tag="et")
                    nc.scalar.activation(et[:], kt_ps[:], EXP)
                    eq = esb.tile([128, BT], FP32R, tag="eq")
                    nc.scalar.activation(eq[:], qt_ps[:], EXP)

                    # softmax over head groups (partition dim) via pool-matmuls
                    s_ps = scr.tile([_H, BT], FP32, tag="scr")
                    nc.tensor.matmul(s_ps[:], ph_sb[:], eq[:], start=True,
                                     stop=True)
                    sr = srp.tile([_H, BT], FP32, tag="sr")
                    nc.vector.reciprocal_approx_fast(sr[:], s_ps[:])
                    bq_ps = scr.tile([128, BT], FP32, tag="scr")
                    nc.tensor.matmul(bq_ps[:], pht_sb[:], sr[:], start=True,
                                     stop=True)
                    nc.vector.tensor_mul(
                        qst_sb[:, k * BT:(k + 1) * BT], eq[:], bq_ps[:]
                    )

                    # transpose E^T back to [t, L], mask, accumulate KX/Ksum
                    e_ps = scr.tile([128, BT], FP32R, tag="scr")
                    for i in range(NT):
                        nc.tensor.transpose(
                            e_ps[:, i * 128:(i + 1) * 128],
                            et[:, i * 128:(i + 1) * 128],
                            id_sb[:],
                        )
                    for i in range(NT):
                        e2 = e2p.tile([128, 128], BF16, tag="e2")
                        j = k * NT + i
                        nc.vector.tensor_scalar_mul(
                            e2[:], e_ps[:, i * 128:(i + 1) * 128],
                            ms_sb[:, j:j + 1],
                        )
                        first = (k == 0 and i == 0)
                        last = (k == NBLK - 1 and i == NT - 1)
                        nc.tensor.matmul(kx_ps[:, 0:512], e2[:],
                                         xts[i][:, 0:512],
                                         start=first, stop=last)
                        nc.tensor.matmul(kx_ps[:, 512:1024], e2[:],
                                         xts[i][:, 512:1024],
                                         start=first, stop=last)
                        nc.tensor.matmul(ks_ps[:], e2[:], on_sb[:],
                                         start=first, stop=last)

                # pack state for the collective
                nc.vector.tensor_copy(kxp_sb[:, 0:512], kx_ps[:, 0:512])
                nc.scalar.copy(kxp_sb[:, 512:1024], kx_ps[:, 512:1024])
                nc.vector.tensor_copy(kxp_sb[:, 1024:1025], ks_ps[:, 0:1])
                nc.vector.memset(kxp_sb[:, 1025:1032], 0.0)

            nc.sync.dma_start(out=ar_in[:], in_=kxp_sb[:])
            nc.gpsimd.collective_compute(
                "AllReduce",
                mybir.AluOpType.add,
                replica_groups=[[0, 1], [2, 3], [4, 5], [6, 7]],
                ins=[ar_in.opt()],
                outs=[ar_out.opt()],
            )
            nc.sync.dma_start(out=kxr_sb[:], in_=ar_out[:])

            # ---------------- Phase B: G = blockdiag(KXn @ Wv) @ o_proj ----
            with (
                tc.tile_pool(name="bsb", bufs=2) as bsb,
                tc.tile_pool(name="bsb1", bufs=1) as bsb1,
                tc.tile_pool(name="bps_small", bufs=2, space="PSUM") as bpss,
                tc.tile_pool(name="bps_big", bufs=2, space="PSUM") as bpsb,
            ):
                rk = bsb1.tile([128, 1], FP32)
                nc.vector.reciprocal_approx_fast(rk[:], kxr_sb[:, 1024:1025])
                kxn = bsb1.tile([128, _D], FP32R)
                nc.vector.tensor_scalar_mul(kxn[:], kxr_sb[:, 0:1024], rk[:])

                kxnt = []
                for c in range(DC):
                    tp = bpss.tile([128, 128], FP32R, tag="btp")
                    nc.tensor.transpose(tp[:], kxn[:, c * 128:(c + 1) * 128],
                                        id_sb[:])
                    t_sb = bsb.tile([128, 128], FP32R, tag="bts")
                    nc.vector.tensor_copy(t_sb[:], tp[:])
                    kxnt.append(t_sb)

                kv_ps = bpsb.tile([128, _D], FP32, tag="big")
                for c in range(DC):
                    nc.tensor.matmul(
                        kv_ps[:, 0:512], kxnt[c][:],
                        wv_sb[:, c * _D:c * _D + 512],
                        start=(c == 0), stop=(c == DC - 1))
                    nc.tensor.matmul(
                        kv_ps[:, 512:1024], kxnt[c][:],
                        wv_sb[:, c * _D + 512:(c + 1) * _D],
                        start=(c == 0), stop=(c == DC - 1))

                # block-diagonal extract via 0/1 mask multiply (f32)
                kvbd = bsb1.tile([128, _D], FP32)
                nc.vector.tensor_mul(kvbd[:], kv_ps[:], bdm_sb[:])
                kvbdt = []
                for c in range(DC):
                    tp = bpss.tile([128, 128], FP32, tag="btpf")
                    nc.tensor.transpose(tp[:], kvbd[:, c * 128:(c + 1) * 128],
                                        id_sb[:].bitcast(FP32))
                    t_sb = bsb.tile([128, 128], FP32R, tag="btsf")
                    nc.vector.tensor_copy(t_sb[:], tp[:])
                    kvbdt.append(t_sb)

                g_ps = bpsb.tile([128, _D], FP32, tag="big")
                for c in range(DC):
                    nc.tensor.matmul(
                        g_ps[:, 0:512], kvbdt[c][:],
                        op_sb[:, c * _D:c * _D + 512],
                        start=(c == 0), stop=(c == DC - 1))
                    nc.tensor.matmul(
                        g_ps[:, 512:1024], kvbdt[c][:],
                        op_sb[:, c * _D + 512:(c + 1) * _D],
                        start=(c == 0), stop=(c == DC - 1))
                nc.vector.tensor_copy(g_sb[:, 0:512], g_ps[:, 0:512])
                nc.scalar.copy(g_sb[:, 512:1024], g_ps[:, 512:1024])

            # ---------------- Phase C: out = Qs @ G ------------------------
            with (
                tc.tile_pool(name="osb", bufs=3) as osb,
                tc.tile_pool(name="ops", bufs=2, space="PSUM") as ops,
            ):
                for i in range(TLOC // 128):
                    o_ps = ops.tile([128, _D], FP32, tag="ops")
                    lhs = qst_sb[:, i * 128:(i + 1) * 128]
                    nc.tensor.matmul(o_ps[:, 0:512], lhs, g_sb[:, 0:512],
                                     start=True, stop=True)
                    nc.tensor.matmul(o_ps[:, 512:1024], lhs,
                                     g_sb[:, 512:1024], start=True, stop=True)
                    ot = osb.tile([128, _D], FP32, tag="osb")
                    if i % 2 == 0:
                        nc.vector.tensor_copy(ot[:], o_ps[:])
                    else:
                        nc.scalar.copy(ot[:], o_ps[:])
                    nc.sync.dma_start(out=out.ap()[i * 128:(i + 1) * 128, :],
                                      in_=ot[:])

    nc.compile()
    return nc


def _host_inputs(X, attention_mask, Wk, Wq, Wv, o_proj):
    import ml_dtypes

    BF = ml_dtypes.bfloat16
    X = np.asarray(X, dtype=np.float32)
    mask = np.asarray(attention_mask, dtype=np.float32)
    Wk = np.asarray(Wk, dtype=np.float32)
    Wq = np.asarray(Wq, dtype=np.float32)
    Wv = np.asarray(Wv, dtype=np.float32)
    o_proj = np.asarray(o_proj, dtype=np.float32)

    wk_r = np.ascontiguousarray(
        Wk.reshape(DC, 128, _L).transpose(1, 0, 2).reshape(128, DC * _L)
    ).astype(BF)
    wq_r = np.ascontiguousarray(
        Wq.reshape(DC, 128, _L).transpose(1, 0, 2).reshape(128, DC * _L)
    ).astype(BF)
    wv_r = np.ascontiguousarray(
        Wv.reshape(DC, 128, _D).transpose(1, 0, 2).reshape(128, DC * _D))
    op_r = np.ascontiguousarray(
        o_proj.reshape(DC, 128, _D).transpose(1, 0, 2).reshape(128, DC * _D))
    ident = np.eye(128, dtype=np.float32)
    ph_m = np.zeros((128, _H), dtype=np.float32)
    for hh in range(_H):
        ph_m[hh * (_L // _H):(hh + 1) * (_L // _H), hh] = 1.0
    pht_m = np.ascontiguousarray(ph_m.T)
    ones2 = np.ones((128, 2), dtype=BF)
    bdm_m = np.zeros((128, _D), dtype=np.float32)
    for hh in range(_H):
        bdm_m[hh * (_L // _H):(hh + 1) * (_L // _H),
              hh * (_D // _H):(hh + 1) * (_D // _H)] = 1.0

    Xbf = X.astype(BF)
    in_maps = []
    for core in range(NCORES):
        b, half = core // 2, core % 2
        xsh = np.ascontiguousarray(Xbf[b, half * TLOC:(half + 1) * TLOC, :])
        xsth = np.ascontiguousarray(xsh.T)
        msh = np.ascontiguousarray(
            mask[b, half * TLOC:(half + 1) * TLOC]
            .reshape(TLOC // 128, 128).T)
        in_maps.append({
            "xs": xsh, "xst": xsth, "ms": msh, "wk": wk_r, "wq": wq_r,
            "wv": wv_r, "op": op_r, "ident": ident, "ph": ph_m,
            "pht": pht_m, "ones2": ones2, "bdm": bdm_m,
        })
    return in_maps


def _run(in_maps, trace=False):
    from concourse.bass_utils import run_bass_kernel_spmd

    if "nc" not in _cache:
        _cache["nc"] = _build()
    return run_bass_kernel_spmd(
        _cache["nc"], in_maps, list(range(NCORES)), trace=trace)


def kernel(X, attention_mask, Wk, Wq, Wv, o_proj, n_heads=16):
    in_maps = _host_inputs(X, attention_mask, Wk, Wq, Wv, o_proj)
    res = _run(in_maps)
    out = np.empty((_B, _T, _D), dtype=np.float32)
    for core in range(NCORES):
        b, half = core // 2, core % 2
        out[b, half * TLOC:(half + 1) * TLOC, :] = res.results[core]["out"]
    return out
